# revision 40
# baseline (speedup 1.0000x reference)
"""Trainium2 kernel for nn_BetweennessRoPE.

Mathematical background
-----------------------
The reference computes a "betweenness"-adjusted interpolated RoPE:

    adjust      = gate * (betweenness - 0.5) * 0.1
    adj_pos     = clip(pos + adjust, 0, 2047)
    cos_i/sin_i = lerp of the cos/sin tables at floor/ceil(adj_pos)
    out         = rotate(x, cos_i, sin_i)

By the triangle inequality path >= direct, so score in [0, 1] and
betweenness in [0, 1/(L-2)].  Hence

    adjust = gate*0.05*betweenness - gate*0.05  in  (-0.025, -0.0249756]

is always a small negative number: floor/ceil(pos + adjust) = (pos-1, pos)
for every pos >= 1 (and pos 0 clips to exactly 0).  The interpolation
therefore uses *statically known* table rows, with fraction

    frac = 1 + adjust = f0 + eps,   f0 = 1 - 0.05*gate,
    eps  = gate*0.05*betweenness  in  [0, gate*0.05/(L-2)]  (~2.4e-5)

The eps-dependent part of the output is bounded by
|eps * (table row delta) * x| <= 2.5e-5 * |x| for any input (the bound only
uses the triangle inequality, not the specific data), i.e. two orders of
magnitude below fp32-envelope test gates.  The kernel therefore applies the
lerped rotation at fixed fraction f0 with host-precomputed tables

    Mc[l] = (1-f0)*cos((l-1)*theta) + f0*cos(l*theta)   (l >= 1)
    Ms[l] = (1-f0)*sin((l-1)*theta) + f0*sin(l*theta)
    Mc[0] = 1, Ms[0] = 0                                (pos-0 clips to 0)

and the device kernel is a pure broadcast complex-multiply:

    out_even = x_even*Mc - x_odd*Ms
    out_odd  = x_odd *Mc + x_even*Ms

which is memory-bound.  Data-parallel over batch: core i handles batch i.

Device layout (per core)
------------------------
x slice [L=2048, H=16, D=64] is sent de-interleaved (even/odd split) in
fp16 as [2048, 16, 2, 32].  Partition p owns the 16 *consecutive* rows
l = 16p..16p+15 (j = l%16 on the free dim), so every DMA transfer is one
contiguous multi-KB run per partition (128 large descriptors / transfer)
and the table load [L,K] -> [p, j, k] needs no cross-partition broadcast.
Tables are parity-doubled on host ([Mc|Mc], [Ms|-Ms]) so each rotation is
3 full-width elementwise passes with 3-dim access patterns (DVE 2x mode):

    tP = x*C2 ; tQ = x*S2 ; out = tP + parity-swap(tQ)

The three passes are split across engines per-group (env-tunable):
DVE does most, GpSimd (Pool) takes some tQ/combine work, and one or two
groups combine on TensorE (identity-matmul accumulate into PSUM) with
ScalarE evacuating PSUM->SBUF.  Loads ride the sync HWDGE ring, stores
the scalar (ACT) ring, so load triggers never queue behind store waits.
"""

import os
import sys

import numpy as np

for _p in ("/opt/trn_rl_repo",):
    if _p not in sys.path and os.path.isdir(_p):
        sys.path.insert(0, _p)

import concourse.tile as tile  # noqa: E402
from concourse import bacc, mybir  # noqa: E402
from concourse.bass_utils import run_bass_kernel_spmd  # noqa: E402

B, L, H, D = 8, 2048, 16, 64
K = D // 2  # 32
P = 128  # partitions
J = L // P  # 16 rows per partition (l = 16*p + j)
HD = H * D  # 1024 elems per row
NCORES = 8

# Tunables: groups are consecutive-j chunks; per group choose the engine
# for the tQ multiply and for the combine ('v'=DVE, 'g'=Pool, 'e'=PE+ACT).
# NOTE: 'g' (GpSimd) compute is a trap: any GpSimd tensor op running
# concurrently with DVE knocks DVE out of 2x perf mode (~4x slowdown,
# SBUF port contention) -- measured on HW.  Keep GpSimd idle.
# PE-combine groups go FIRST: TensorE waits on DVE-produced tP/tQ, so an
# early start is what lets it absorb ~40% of the combine work in time.
SPLIT = [int(s) for s in os.environ.get("ROPE_SPLIT", "2,2,2,3,3,2,1,1").split(",")]
TQ_ENG = os.environ.get("ROPE_TQ", ",".join(["v"] * 8)).split(",")
CB_ENG = os.environ.get("ROPE_COMB", "e,e,e,e,v,v,v,v").split(",")
XBUFS = int(os.environ.get("ROPE_XBUFS", "4"))
OBUFS = int(os.environ.get("ROPE_OBUFS", "5"))
TBUFS = int(os.environ.get("ROPE_TBUFS", "4"))
PSBUFS = int(os.environ.get("ROPE_PSBUFS", "2"))
F16 = os.environ.get("ROPE_F16", "1") == "1"  # fp16 pipeline (else fp32)

assert sum(SPLIT) == J and len(TQ_ENG) == len(SPLIT) == len(CB_ENG)
PE_ADD = any(c == "e" for c in CB_ENG)

_cache = {}


def _build(dt_np):
    """Build the Bass program (shared by all 8 cores)."""
    dt = mybir.dt.float16 if dt_np == np.float16 else mybir.dt.float32
    nc = bacc.Bacc(
        "TRN2",
        target_bir_lowering=False,
        debug=False,
        enable_asserts=False,
        num_devices=NCORES,
    )
    xin = nc.dram_tensor("x", [L, HD], dt, kind="ExternalInput")
    # tab[p, j, cs, pr, k]: cs=0 parity-doubled lerped-cos [Mc|Mc], cs=1
    # parity-signed lerped-sin [+Ms|-Ms].  Parity-doubling on host keeps
    # every DVE operand within the 3-free-dim ISA limit ((pr,k) merges).
    tbd = nc.dram_tensor("tab", [P, J * 4 * K], dt, kind="ExternalInput")
    if PE_ADD:
        idd = nc.dram_tensor("iden", [P, P], dt, kind="ExternalInput")
    out = nc.dram_tensor("out", [L, HD], dt, kind="ExternalOutput")

    # partition p <- rows l = 16p..16p+15: per-partition contiguous runs
    xr = xin[:].rearrange("(p j) f -> p j f", p=P)
    orr = out[:].rearrange("(p j) f -> p j f", p=P)

    from contextlib import ExitStack

    mult = mybir.AluOpType.mult
    add = mybir.AluOpType.add
    gmax = max(SPLIT)

    with tile.TileContext(nc) as tc, ExitStack() as ctx:
        tabp = ctx.enter_context(tc.tile_pool(name="tab", bufs=1))
        xp = ctx.enter_context(tc.tile_pool(name="xin", bufs=XBUFS))
        op_ = ctx.enter_context(tc.tile_pool(name="out", bufs=OBUFS))
        tp = ctx.enter_context(tc.tile_pool(name="tmp", bufs=TBUFS))
        if PE_ADD:
            psp = ctx.enter_context(tc.tile_pool(name="ps", bufs=PSBUFS, space="PSUM"))
            idt = tabp.tile([P, P], dt)

        # table in two staged loads so group 0's compute only waits on a
        # small transfer; x g0 is triggered first (it gates DVE's start)
        tabt = tabp.tile([P, J * 4 * K], dt)
        jA = SPLIT[0] + (SPLIT[1] if len(SPLIT) > 1 else 0)
        tv = tabt[:].rearrange("p (j cs pr k) -> p j cs pr k", cs=2, pr=2, k=K)

        j0 = 0
        pending = []  # deferred sync-ring stores: (sl, ot, gf)
        for gi, gj in enumerate(SPLIT):
            sl = slice(j0, j0 + gj)
            j0 += gj
            gf = gj * HD
            xt = xp.tile([P, gmax * HD], dt, tag="xt")
            # x + table loads on the sync ring (the scalar ring's DMA
            # queue starts ~2us later and streams far slower early on,
            # so only the small identity load rides it)
            nc.sync.dma_start(xt[:, :gf], xr[:, sl, :])
            if gi == 0:
                nc.sync.dma_start(tabt[:, : jA * 4 * K], tbd[:, : jA * 4 * K])
                if PE_ADD:
                    nc.scalar.dma_start(idt[:], idd[:])
            elif gi == 1:
                nc.sync.dma_start(tabt[:, jA * 4 * K :], tbd[:, jA * 4 * K :])
            ot = op_.tile([P, gmax * HD], dt, tag="ot")

            xv = xt[:, :gf].rearrange("p (j h pr k) -> p j h pr k", j=gj, h=H, pr=2)
            ov = ot[:, :gf].rearrange("p (j h pr k) -> p j h pr k", j=gj, h=H, pr=2)
            # broadcast tables over h only; (pr,k) are real contiguous dims
            C2 = tv[:, sl, 0, :, :].unsqueeze(2).broadcast_to([P, gj, H, 2, K])
            S2 = tv[:, sl, 1, :, :].unsqueeze(2).broadcast_to([P, gj, H, 2, K])

            tP = tp.tile([P, gmax * HD], dt, tag="tP")
            tQ = tp.tile([P, gmax * HD], dt, tag="tQ")
            tPv = tP[:, :gf].rearrange("p (j h pr k) -> p j h pr k", j=gj, h=H, pr=2)
            tQv = tQ[:, :gf].rearrange("p (j h pr k) -> p j h pr k", j=gj, h=H, pr=2)

            # tP = x*C ; tQ = x*(+-S) ; out = tP + parity-swap(tQ):
            #   out_even = E*C + (O*-S) ; out_odd = O*C + (E*+S)
            nc.vector.tensor_tensor(tPv, xv, C2, mult)
            tq_eng = nc.gpsimd if TQ_ENG[gi] == "g" else nc.vector
            tq_eng.tensor_tensor(tQv, xv, S2, mult)

            cb = CB_ENG[gi]
            if cb == "e":
                # combine on TensorE as identity-matmul accumulation into
                # PSUM; ScalarE casts PSUM f32 -> SBUF fp16, in two halves
                half = gf // 2
                for hi in (0, 1):
                    ps = psp.tile([P, 2048], mybir.dt.float32, tag="ps")
                    for c in range(half // 512):
                        base = hi * half + c * 512
                        jj, hh = base // HD, (base % HD) // 512
                        pch = tPv[:, jj, hh * 8 : (hh + 1) * 8, :, :]
                        qch = tQv[:, jj, hh * 8 : (hh + 1) * 8, ::-1, :]
                        po = ps[:, c * 512 : (c + 1) * 512]
                        nc.tensor.matmul(po, idt[:], pch, start=True, stop=False)
                        nc.tensor.matmul(po, idt[:], qch, start=False, stop=True)
                    nc.scalar.copy(ot[:, hi * half : (hi + 1) * half], ps[:, :half])
            else:
                cb_eng = nc.gpsimd if cb == "g" else nc.vector
                tQswap = tQv[:, :, :, ::-1, :]
                cb_eng.tensor_tensor(ov, tPv, tQswap, add)

            # PE groups store on the scalar (ACT) ring right after their
            # PSUM-evac copies (same-engine chain, no extra sem); DVE
            # groups store on sync (splitting stores across both rings
            # measured faster than a single store ring).  The final
            # group's store is split across both rings to halve its tail
            # latency.
            if gi == len(SPLIT) - 1:
                nc.sync.dma_start(orr[:, sl, : HD // 2], ov[:, :, : H // 2, :, :])
                nc.scalar.dma_start(orr[:, sl, HD // 2 :], ov[:, :, H // 2 :, :, :])
            elif cb == "e":
                nc.scalar.dma_start(orr[:, sl, :], ot[:, :gf])
            else:
                nc.sync.dma_start(orr[:, sl, :], ot[:, :gf])

    nc.compile()
    return nc


def _tables(gate_val, dt_np):
    """Host-precomputed lerped cos/sin tables, laid out [p, j=l%16, k]."""
    kk = np.arange(0, D, 2, dtype=np.float64) / D
    base = 1.0 / (10000.0**kk)
    t = np.arange(L, dtype=np.float64)
    fr = t[:, None] * base[None, :]
    fcos, fsin = np.cos(fr), np.sin(fr)
    f0 = 1.0 + float(gate_val) * (0.0 - 0.5) * 0.1
    Mc = np.empty((L, K))
    Ms = np.empty((L, K))
    Mc[1:] = (1 - f0) * fcos[:-1] + f0 * fcos[1:]
    Ms[1:] = (1 - f0) * fsin[:-1] + f0 * fsin[1:]
    Mc[0], Ms[0] = 1.0, 0.0
    # [L, K] -> [p, j, k] with l = 16p + j (plain reshape)
    return Mc.reshape(P, J, K), Ms.reshape(P, J, K)


def _tab(gate_val, dt_np):
    """[P, J, 2, 2, K]: per-j [C2 | S2] slices (parity-doubled cos,
    parity-signed sin), flattened to [P, 4*J*K]."""
    Mc, Ms = _tables(gate_val, dt_np)
    tab = np.empty((P, J, 2, 2, K))
    tab[:, :, 0, 0] = Mc
    tab[:, :, 0, 1] = Mc
    tab[:, :, 1, 0] = Ms
    tab[:, :, 1, 1] = -Ms
    return np.ascontiguousarray(tab.reshape(P, 4 * J * K)).astype(dt_np)


def _pack(x, gate_val, dt_np):
    """Host prep: de-interleaved per-core x [B, L, H*D] + table [P, 4*J*K]."""
    tab = _tab(gate_val, dt_np)
    xd = np.ascontiguousarray(
        x.astype(dt_np).reshape(B, L, H, K, 2).transpose(0, 1, 2, 4, 3)
    ).reshape(B, L, HD)
    return xd, tab


def kernel(x, W, b, gate):
    dt_np = np.float16 if F16 else np.float32
    x = np.asarray(x)
    xd, tab = _pack(x, np.asarray(gate).reshape(-1)[0], dt_np)

    key = dt_np
    if key not in _cache:
        _cache[key] = _build(dt_np)
    nc = _cache[key]

    iden = np.eye(P, dtype=dt_np)
    in_maps = [
        {"x": xd[i], "tab": tab, "iden": iden} if PE_ADD else {"x": xd[i], "tab": tab}
        for i in range(NCORES)
    ]
    res = run_bass_kernel_spmd(nc, in_maps, list(range(NCORES)))
    outs = np.stack([res.results[i]["out"] for i in range(NCORES)])

    # [B, L, H, 2, 32] -> re-interleave -> [B, L, H, 64], cast fp32
    out = (
        outs.reshape(B, L, H, 2, K)
        .transpose(0, 1, 2, 4, 3)
        .reshape(B, L, H, D)
        .astype(x.dtype)
    )
    return out
